# revision 10
# baseline (speedup 1.0000x reference)
"""Multi-head self-attention on 8 Trainium2 NeuronCores.

Sharding: core i handles batch b = i // 4 and head-group g = i % 4
(4 of 16 heads).  Tensor-parallel over heads for the QKV/attention/output
projection, data-parallel over batch.  Each core produces a partial
output (its head-group's slice of the final projection); the all-reduce
over the 4 head-group partials per batch happens on the host after the
gather, together with adding the output bias exactly once (only g == 0
cores receive the real bo).

Device layout notes:
  - Host pre-transposes x to xT and pre-packs every tensor into the
    exact [128, F] SBUF image the kernel DMAs, so the device never
    transposes anything.
  - All matmul operands are float32r (full-rate single-pass fp32 on the
    PE array, ~1e-4 relative rounding); PSUM accumulation stays fp32.
  - Attention uses the scores-transposed layout: scoresT[k, q] tiles so
    that exp(scoresT) is directly the PV matmul's moving operand, and
    the softmax row sums come for free from a ones-column appended to
    the stationary V tile.
"""

import numpy as np

B, S, D = 2, 2048, 1024
H, DH = 16, 64
NCORE = 8
TP = 4  # head-group shards per batch
HPC = H // TP  # heads per core
DHC = HPC * DH  # 256 = per-core slice of the model dim

_cache = {}


def _build():
    import concourse.bacc as bacc
    import concourse.mybir as mybir
    import concourse.tile as tile

    F32 = mybir.dt.float32
    F32R = mybir.dt.float32r
    EXP = mybir.ActivationFunctionType.Exp
    MULT = mybir.AluOpType.mult
    ADD = mybir.AluOpType.add

    nc = bacc.Bacc("TRN2", target_bir_lowering=False, debug=False, num_devices=NCORE)

    # DRAM I/O (all images pre-packed on host)
    xt = nc.dram_tensor("xt", [4, 128, 4096], F32R, kind="ExternalInput").ap()
    wq = nc.dram_tensor("wq", [128, 2048], F32R, kind="ExternalInput").ap()
    wk = nc.dram_tensor("wk", [128, 2048], F32R, kind="ExternalInput").ap()
    wv = nc.dram_tensor("wv", [128, 2048], F32R, kind="ExternalInput").ap()
    wo = nc.dram_tensor("wo", [128, 2048], F32R, kind="ExternalInput").ap()
    bqk = nc.dram_tensor("bqk", [128, 4], F32, kind="ExternalInput").ap()
    bv = nc.dram_tensor("bv", [1, DHC], F32, kind="ExternalInput").ap()
    bo = nc.dram_tensor("bo", [1, D], F32, kind="ExternalInput").ap()
    y = nc.dram_tensor("y", [S, D], F32, kind="ExternalOutput").ap()

    with tile.TileContext(nc) as tc:
        with (
            tc.tile_pool(name="const", bufs=1) as pc,
            tc.tile_pool(name="w", bufs=1) as pw,
            tc.tile_pool(name="x", bufs=2) as px,
            tc.tile_pool(name="qkv", bufs=1) as pqkv,
            tc.tile_pool(name="pt", bufs=4) as ppt,
            tc.tile_pool(name="r", bufs=2) as pr,
            tc.tile_pool(name="out", bufs=3) as pout,
            tc.tile_pool(name="mm", bufs=1, space="PSUM") as pmm,
            tc.tile_pool(name="pse", bufs=1, space="PSUM") as pse,
        ):
            # ---- constants / weights ----
            bqk_sb = pc.tile([128, 4], F32)
            nc.sync.dma_start(out=bqk_sb[:], in_=bqk[:])
            bv_sb = pc.tile([128, DHC], F32)
            nc.sync.dma_start(out=bv_sb[:], in_=bv.to_broadcast((128, DHC)))
            bo_sb = pc.tile([128, D], F32)
            nc.sync.dma_start(out=bo_sb[:], in_=bo.to_broadcast((128, D)))

            wq_sb = pw.tile([128, 2048], F32R, tag="wq")
            wk_sb = pw.tile([128, 2048], F32R, tag="wk")
            wv_sb = pw.tile([128, 2048], F32R, tag="wv")
            wo_sb = pw.tile([128, 2048], F32R, tag="wo")
            nc.sync.dma_start(out=wq_sb[:], in_=wq[:])
            nc.sync.dma_start(out=wk_sb[:], in_=wk[:])
            nc.sync.dma_start(out=wv_sb[:], in_=wv[:])
            nc.sync.dma_start(out=wo_sb[:], in_=wo[:])

            # ---- persistent activations ----
            # qT/kT: [dh(2 heads), seq] per head-pair g2, g2-major columns
            qT = pqkv.tile([128, 4096], F32R, tag="qT")
            kT = pqkv.tile([128, 4096], F32R, tag="kT")
            # v: per seq-tile st: 4 heads x (64 v-cols + ones col)
            vsb = pqkv.tile([128, 16 * (DH + 1) * HPC], F32R, tag="v")
            # (columns: st*260 + h*65 + c); fill the ones columns
            ones_sb = pc.tile([128, 1], F32, tag="ones")
            nc.vector.memset(ones_sb[:], 1.0)
            nc.vector.tensor_copy(
                vsb[:].rearrange("p (st h c2) -> p st h c2", st=16, h=HPC)[
                    :, :, :, DH : DH + 1
                ],
                ones_sb[:].to_broadcast((128, 16, HPC, 1)),
            )
            embT = pqkv.tile([128, 4096], F32R, tag="embT")

            # ---- projections, streamed over 4 seq-chunks of 512 ----
            xc_tiles = []
            for c in range(4):
                xc = px.tile([128, 4096], F32R, tag="xc")
                nc.sync.dma_start(out=xc[:], in_=xt[c])
                xc_tiles.append(xc)

            # proj/O-proj psums rotate over the two score tags (s0/s1) so the
            # PSUM pool has exactly 4 mm banks + 4 pse banks at all times
            psn = [0]

            def mm_tile():
                psn[0] ^= 1
                return pmm.tile(
                    [128, 1024], F32, tag=f"s{psn[0]}", name=f"ps{psn[0]}_{nc.next_id()}"
                )

            for scp in range(2):  # pairs of 512-seq chunks
                chunks = (2 * scp, 2 * scp + 1)
                # K and Q projections -> kT/qT (transposed layout)
                for proj, wsb, tsb, boff in (
                    ("k", wk_sb, kT, 2),
                    ("q", wq_sb, qT, 0),
                ):
                    for g2 in range(2):
                        ps = mm_tile()
                        for ci, c in enumerate(chunks):
                            for kt in range(8):
                                nc.tensor.matmul(
                                    ps[:, ci * 512 : ci * 512 + 512],
                                    wsb[:, kt * 256 + g2 * 128 : kt * 256 + g2 * 128 + 128],
                                    xc_tiles[c][:, kt * 512 : kt * 512 + 512],
                                    start=(kt == 0),
                                    stop=(kt == 7),
                                )
                        nc.vector.tensor_scalar_add(
                            tsb[:, g2 * 2048 + scp * 1024 : g2 * 2048 + scp * 1024 + 1024],
                            ps[:],
                            bqk_sb[:, boff + g2 : boff + g2 + 1],
                        )
                # V projection -> natural layout [seq, dh] with ones cols kept
                for ci, c in enumerate(chunks):
                    ps = mm_tile()
                    for stl in range(4):  # 4 seq-tiles of 128 in this chunk
                        for kt in range(8):
                            nc.tensor.matmul(
                                ps[:, stl * 256 : stl * 256 + 256],
                                xc_tiles[c][:, kt * 512 + stl * 128 : kt * 512 + stl * 128 + 128],
                                wv_sb[:, kt * 256 : kt * 256 + 256],
                                start=(kt == 0),
                                stop=(kt == 7),
                            )
                    for stl in range(4):
                        st = c * 4 + stl
                        vo = vsb[:, st * 260 : st * 260 + 260].rearrange(
                            "p (h c2) -> p h c2", h=HPC
                        )[:, :, 0:DH]
                        nc.vector.tensor_tensor(
                            out=vo,
                            in0=ps[:, stl * 256 : stl * 256 + 256].rearrange(
                                "p (h c2) -> p h c2", h=HPC
                            ),
                            in1=bv_sb[:].rearrange("p (h c2) -> p h c2", h=HPC),
                            op=ADD,
                        )

            # ---- attention over (head-pair g2, 1024-query chunk) units ----
            # The two heads of a pair live on partition halves of qT/kT, so
            # each kt's two QK matmuls are K=64 row-group-packed (concurrent
            # in the PE array); PV stays K=128 with the ones column giving
            # softmax sums in psum row 64.
            for g2 in range(2):
                for j in range(2):
                    jo = g2 * 2048 + j * 1024
                    pacc = [
                        pse.tile([65, 1024], F32, tag=f"pse{hh}", name=f"pse{hh}_{g2}_{j}")
                        for hh in range(2)
                    ]
                    for kt in range(16):
                        ko = g2 * 2048 + kt * 128
                        ps_s = [
                            pmm.tile([128, 1024], F32, tag=f"s{hh}", name=f"s{hh}_{g2}_{j}_{kt}")
                            for hh in range(2)
                        ]
                        for qc in (0, 512):
                            for hh in range(2):
                                plo = hh * 64
                                nc.tensor.matmul(
                                    ps_s[hh][:, qc : qc + 512],
                                    kT[plo : plo + 64, ko : ko + 128],
                                    qT[plo : plo + 64, jo + qc : jo + qc + 512],
                                    start=True,
                                    stop=True,
                                )
                        pts = []
                        for hh in range(2):
                            pt = ppt.tile([128, 1024], F32R, tag="pt", name=f"pt{hh}_{g2}_{j}_{kt}")
                            nc.scalar.activation(pt[:], ps_s[hh][:], EXP, scale=0.125)
                            pts.append(pt)
                        for hh in range(2):
                            h = 2 * g2 + hh
                            for qc in (0, 512):
                                nc.tensor.matmul(
                                    pacc[hh][:, qc : qc + 512],
                                    vsb[:, kt * 260 + h * 65 : kt * 260 + h * 65 + 65],
                                    pts[hh][:, qc : qc + 512],
                                    start=(kt == 0),
                                    stop=(kt == 15),
                                )
                    for hh in range(2):
                        plo = hh * 64
                        r = pr.tile([1, 1024], F32, tag="r", name=f"r_{g2}_{j}_{hh}")
                        nc.vector.reciprocal(r[:], pacc[hh][64:65, :])
                        rb = pr.tile([64, 1024], F32, tag="rb", name=f"rb_{g2}_{j}_{hh}")
                        nc.gpsimd.partition_broadcast(rb[:], r[:])
                        nc.vector.tensor_tensor(
                            out=embT[plo : plo + 64, jo : jo + 1024],
                            in0=pacc[hh][0:64, :],
                            in1=rb[:],
                            op=MULT,
                        )

            # ---- output projection (partial over this core's head slice) ----
            for qt in range(16):
                ps_o = mm_tile()
                for do in range(2):
                    for g2 in range(2):
                        nc.tensor.matmul(
                            ps_o[:, do * 512 : do * 512 + 512],
                            embT[:, g2 * 2048 + qt * 128 : g2 * 2048 + qt * 128 + 128],
                            wo_sb[:, g2 * 1024 + do * 512 : g2 * 1024 + do * 512 + 512],
                            start=(g2 == 0),
                            stop=(g2 == 1),
                        )
                ot = pout.tile([128, 1024], F32, tag="ot")
                nc.vector.tensor_tensor(out=ot[:], in0=ps_o[:], in1=bo_sb[:], op=ADD)
                nc.sync.dma_start(out=y[qt * 128 : qt * 128 + 128, :], in_=ot[:])

    nc.compile()
    return nc


def _pack_inputs(x, Wq, bq, Wk, bk, Wv, bv, Wo, bo):
    """Per-core host-side sharding into the exact DMA images."""

    def img_w(Wslice):  # [1024, 256] -> [128, 8*256]
        return np.ascontiguousarray(
            Wslice.reshape(8, 128, DHC).transpose(1, 0, 2).reshape(128, 8 * DHC)
        )

    in_maps = []
    for i in range(NCORE):
        b, g = i // TP, i % TP
        sl = slice(g * DHC, (g + 1) * DHC)
        xT = x[b].T  # [1024, 2048]
        xt_img = np.ascontiguousarray(
            xT.reshape(8, 128, 4, 512).transpose(2, 1, 0, 3).reshape(4, 128, 4096)
        )
        bq_rs = bq[sl].reshape(2, 128).T  # [128, 2]
        bk_rs = bk[sl].reshape(2, 128).T
        bqk_img = np.ascontiguousarray(np.concatenate([bq_rs, bk_rs], axis=1))
        wo_img = np.ascontiguousarray(
            Wo[sl, :].reshape(2, 128, D).transpose(1, 0, 2).reshape(128, 2 * D)
        )
        in_maps.append(
            {
                "xt": xt_img,
                "wq": img_w(Wq[:, sl]),
                "wk": img_w(Wk[:, sl]),
                "wv": img_w(Wv[:, sl]),
                "wo": wo_img,
                "bqk": bqk_img,
                "bv": np.ascontiguousarray(bv[sl].reshape(1, DHC)),
                "bo": np.ascontiguousarray(
                    (bo if g == 0 else np.zeros_like(bo)).reshape(1, D)
                ),
            }
        )
    return in_maps


def kernel(x, Wq, bq, Wk, bk, Wv, bv, Wo, bo, _trace=False):
    from concourse.bass_utils import run_bass_kernel_spmd

    args = [np.asarray(a, dtype=np.float32) for a in (x, Wq, bq, Wk, bk, Wv, bv, Wo, bo)]
    if "nc" not in _cache:
        _cache["nc"] = _build()
    nc = _cache["nc"]

    in_maps = _pack_inputs(*args)
    res = run_bass_kernel_spmd(nc, in_maps, list(range(NCORE)), trace=_trace)
    _cache["last_result"] = res

    out = np.zeros((B, S, D), dtype=np.float32)
    for i in range(NCORE):
        out[i // TP] += res.results[i]["y"]
    return out


# revision 12
# speedup vs baseline: 1.5305x; 1.5305x over previous
"""Multi-head self-attention on 8 Trainium2 NeuronCores.

Sharding: core i handles batch b = i // 4 and head-group g = i % 4
(4 of 16 heads).  Tensor-parallel over heads for the QKV/attention/output
projection, data-parallel over batch.  Each core produces a partial
output (its head-group's slice of the final projection); the all-reduce
over the 4 head-group partials per batch happens on the host after the
gather, together with adding the output bias exactly once (only g == 0
cores receive the real bo).

Device layout notes:
  - Host pre-transposes x to xT and pre-packs every tensor into the
    exact [128, F] SBUF image the kernel DMAs, so the device never
    transposes anything.
  - All matmul operands are float32r (full-rate single-pass fp32 on the
    PE array, ~1e-4 relative rounding); PSUM accumulation stays fp32.
  - Attention uses the scores-transposed layout: scoresT[k, q] tiles so
    that exp(scoresT) is directly the PV matmul's moving operand, and
    the softmax row sums come for free from a ones-column appended to
    the stationary V tile.
"""

import numpy as np

B, S, D = 2, 2048, 1024
H, DH = 16, 64
NCORE = 8
TP = 4  # head-group shards per batch
HPC = H // TP  # heads per core
DHC = HPC * DH  # 256 = per-core slice of the model dim

_cache = {}


def _build():
    import concourse.bacc as bacc
    import concourse.mybir as mybir
    import concourse.tile as tile

    F32 = mybir.dt.float32
    F32R = mybir.dt.float32r
    EXP = mybir.ActivationFunctionType.Exp
    MULT = mybir.AluOpType.mult
    ADD = mybir.AluOpType.add

    nc = bacc.Bacc("TRN2", target_bir_lowering=False, debug=False, num_devices=NCORE)

    # DRAM I/O (all images pre-packed on host)
    xt = nc.dram_tensor("xt", [4, 128, 4096], F32R, kind="ExternalInput").ap()
    wq = nc.dram_tensor("wq", [128, 2048], F32R, kind="ExternalInput").ap()
    wk = nc.dram_tensor("wk", [128, 2048], F32R, kind="ExternalInput").ap()
    wv = nc.dram_tensor("wv", [128, 2048], F32R, kind="ExternalInput").ap()
    wo = nc.dram_tensor("wo", [128, 2048], F32R, kind="ExternalInput").ap()
    bqk = nc.dram_tensor("bqk", [128, 4], F32, kind="ExternalInput").ap()
    bv = nc.dram_tensor("bv", [1, DHC], F32, kind="ExternalInput").ap()
    bo = nc.dram_tensor("bo", [1, D], F32, kind="ExternalInput").ap()
    y = nc.dram_tensor("y", [S, D], F32, kind="ExternalOutput").ap()

    with tile.TileContext(nc) as tc:
        with (
            tc.tile_pool(name="const", bufs=1) as pc,
            tc.tile_pool(name="w", bufs=1) as pw,
            tc.tile_pool(name="x", bufs=2) as px,
            tc.tile_pool(name="qkv", bufs=1) as pqkv,
            tc.tile_pool(name="pt", bufs=4) as ppt,
            tc.tile_pool(name="r", bufs=2) as pr,
            tc.tile_pool(name="out", bufs=3) as pout,
            tc.tile_pool(name="mm", bufs=1, space="PSUM") as pmm,
            tc.tile_pool(name="pse", bufs=2, space="PSUM") as pse,
        ):
            # ---- constants / weights ----
            bqk_sb = pc.tile([128, 4], F32)
            nc.sync.dma_start(out=bqk_sb[:], in_=bqk[:])
            bv_sb = pc.tile([128, DHC], F32)
            nc.sync.dma_start(out=bv_sb[:], in_=bv.to_broadcast((128, DHC)))
            bo_sb = pc.tile([128, D], F32)
            nc.sync.dma_start(out=bo_sb[:], in_=bo.to_broadcast((128, D)))

            wq_sb = pw.tile([128, 2048], F32R, tag="wq")
            wk_sb = pw.tile([128, 2048], F32R, tag="wk")
            wv_sb = pw.tile([128, 2048], F32R, tag="wv")
            wo_sb = pw.tile([128, 2048], F32R, tag="wo")
            nc.sync.dma_start(out=wq_sb[:], in_=wq[:])
            nc.sync.dma_start(out=wk_sb[:], in_=wk[:])
            nc.sync.dma_start(out=wv_sb[:], in_=wv[:])
            nc.sync.dma_start(out=wo_sb[:], in_=wo[:])

            # ---- persistent activations ----
            # qT/kT: [dh(2 heads), seq] per head-pair g2, g2-major columns
            qT = pqkv.tile([128, 4096], F32R, tag="qT")
            kT = pqkv.tile([128, 4096], F32R, tag="kT")
            # v: per seq-tile st: 4 heads x (64 v-cols + ones col)
            vsb = pqkv.tile([128, 16 * (DH + 1) * HPC], F32R, tag="v")
            # (columns: st*260 + h*65 + c); fill the ones columns
            ones_sb = pc.tile([128, 1], F32, tag="ones")
            nc.vector.memset(ones_sb[:], 1.0)
            nc.vector.tensor_copy(
                vsb[:].rearrange("p (st h c2) -> p st h c2", st=16, h=HPC)[
                    :, :, :, DH : DH + 1
                ],
                ones_sb[:].to_broadcast((128, 16, HPC, 1)),
            )
            embT = pqkv.tile([128, 4096], F32R, tag="embT")

            # ---- projections, streamed over 4 seq-chunks of 512 ----
            xc_tiles = []
            for c in range(4):
                xc = px.tile([128, 4096], F32R, tag="xc")
                nc.sync.dma_start(out=xc[:], in_=xt[c])
                xc_tiles.append(xc)

            # proj/O-proj psums rotate over the two score tags (s0/s1) so the
            # PSUM pool has exactly 4 mm banks + 4 pse banks at all times
            psn = [0]

            def mm_tile():
                psn[0] ^= 1
                return pmm.tile(
                    [128, 1024], F32, tag=f"s{psn[0]}", name=f"ps{psn[0]}_{nc.next_id()}"
                )

            for scp in range(2):  # pairs of 512-seq chunks
                chunks = (2 * scp, 2 * scp + 1)
                # K and Q projections -> kT/qT (transposed layout)
                for proj, wsb, tsb, boff in (
                    ("k", wk_sb, kT, 2),
                    ("q", wq_sb, qT, 0),
                ):
                    for g2 in range(2):
                        ps = mm_tile()
                        for ci, c in enumerate(chunks):
                            for kt in range(8):
                                nc.tensor.matmul(
                                    ps[:, ci * 512 : ci * 512 + 512],
                                    wsb[:, kt * 256 + g2 * 128 : kt * 256 + g2 * 128 + 128],
                                    xc_tiles[c][:, kt * 512 : kt * 512 + 512],
                                    start=(kt == 0),
                                    stop=(kt == 7),
                                )
                        nc.vector.tensor_scalar_add(
                            tsb[:, g2 * 2048 + scp * 1024 : g2 * 2048 + scp * 1024 + 1024],
                            ps[:],
                            bqk_sb[:, boff + g2 : boff + g2 + 1],
                        )
                # V projection -> natural layout [seq, dh] with ones cols kept
                for ci, c in enumerate(chunks):
                    ps = mm_tile()
                    for stl in range(4):  # 4 seq-tiles of 128 in this chunk
                        for kt in range(8):
                            nc.tensor.matmul(
                                ps[:, stl * 256 : stl * 256 + 256],
                                xc_tiles[c][:, kt * 512 + stl * 128 : kt * 512 + stl * 128 + 128],
                                wv_sb[:, kt * 256 : kt * 256 + 256],
                                start=(kt == 0),
                                stop=(kt == 7),
                            )
                    for stl in range(4):
                        st = c * 4 + stl
                        vo = vsb[:, st * 260 : st * 260 + 260].rearrange(
                            "p (h c2) -> p h c2", h=HPC
                        )[:, :, 0:DH]
                        nc.vector.tensor_tensor(
                            out=vo,
                            in0=ps[:, stl * 256 : stl * 256 + 256].rearrange(
                                "p (h c2) -> p h c2", h=HPC
                            ),
                            in1=bv_sb[:].rearrange("p (h c2) -> p h c2", h=HPC),
                            op=ADD,
                        )

            # ---- attention over (head-pair g2, 512-query chunk) units ----
            # Per kt, the two heads' scoresT share one [128, 1024] psum tile
            # (h0 in cols 0:512 = bank 0, h1 in cols 512:1024 = bank 1): the
            # two QK matmuls are a row-group-packed K=64 pair (concurrent in
            # the PE array) and a single exp covers both heads.  PV runs per
            # head (K=128, ones column -> softmax sums in psum row 64).
            for g2 in range(2):
                for j in range(4):
                    jo = g2 * 2048 + j * 512
                    pacc = [
                        pse.tile([65, 512], F32, tag=f"pse{hh}", name=f"pse{hh}_{g2}_{j}")
                        for hh in range(2)
                    ]
                    for kt in range(16):
                        ko = g2 * 2048 + kt * 128
                        ps = mm_tile()
                        for hh in range(2):
                            plo = hh * 64
                            nc.tensor.matmul(
                                ps[:, hh * 512 : hh * 512 + 512],
                                kT[plo : plo + 64, ko : ko + 128],
                                qT[plo : plo + 64, jo : jo + 512],
                                start=True,
                                stop=True,
                            )
                        pt = ppt.tile([128, 1024], F32R, tag="pt", name=f"pt_{g2}_{j}_{kt}")
                        nc.scalar.activation(pt[:], ps[:], EXP, scale=0.125)
                        for hh in range(2):
                            h = 2 * g2 + hh
                            nc.tensor.matmul(
                                pacc[hh][:],
                                vsb[:, kt * 260 + h * 65 : kt * 260 + h * 65 + 65],
                                pt[:, hh * 512 : hh * 512 + 512],
                                start=(kt == 0),
                                stop=(kt == 15),
                            )
                    for hh in range(2):
                        plo = hh * 64
                        r = pr.tile([1, 512], F32, tag="r", name=f"r_{g2}_{j}_{hh}")
                        nc.vector.reciprocal(r[:], pacc[hh][64:65, :])
                        rb = pr.tile([64, 512], F32, tag="rb", name=f"rb_{g2}_{j}_{hh}")
                        nc.gpsimd.partition_broadcast(rb[:], r[:])
                        nc.vector.tensor_tensor(
                            out=embT[plo : plo + 64, jo : jo + 512],
                            in0=pacc[hh][0:64, :],
                            in1=rb[:],
                            op=MULT,
                        )

            # ---- output projection (partial over this core's head slice) ----
            for qt in range(16):
                ps_o = mm_tile()
                for do in range(2):
                    for g2 in range(2):
                        nc.tensor.matmul(
                            ps_o[:, do * 512 : do * 512 + 512],
                            embT[:, g2 * 2048 + qt * 128 : g2 * 2048 + qt * 128 + 128],
                            wo_sb[:, g2 * 1024 + do * 512 : g2 * 1024 + do * 512 + 512],
                            start=(g2 == 0),
                            stop=(g2 == 1),
                        )
                ot = pout.tile([128, 1024], F32, tag="ot")
                nc.vector.tensor_tensor(out=ot[:], in0=ps_o[:], in1=bo_sb[:], op=ADD)
                nc.sync.dma_start(out=y[qt * 128 : qt * 128 + 128, :], in_=ot[:])

    nc.compile()
    return nc


def _pack_inputs(x, Wq, bq, Wk, bk, Wv, bv, Wo, bo):
    """Per-core host-side sharding into the exact DMA images."""

    def img_w(Wslice):  # [1024, 256] -> [128, 8*256]
        return np.ascontiguousarray(
            Wslice.reshape(8, 128, DHC).transpose(1, 0, 2).reshape(128, 8 * DHC)
        )

    in_maps = []
    for i in range(NCORE):
        b, g = i // TP, i % TP
        sl = slice(g * DHC, (g + 1) * DHC)
        xT = x[b].T  # [1024, 2048]
        xt_img = np.ascontiguousarray(
            xT.reshape(8, 128, 4, 512).transpose(2, 1, 0, 3).reshape(4, 128, 4096)
        )
        bq_rs = bq[sl].reshape(2, 128).T  # [128, 2]
        bk_rs = bk[sl].reshape(2, 128).T
        bqk_img = np.ascontiguousarray(np.concatenate([bq_rs, bk_rs], axis=1))
        wo_img = np.ascontiguousarray(
            Wo[sl, :].reshape(2, 128, D).transpose(1, 0, 2).reshape(128, 2 * D)
        )
        in_maps.append(
            {
                "xt": xt_img,
                "wq": img_w(Wq[:, sl]),
                "wk": img_w(Wk[:, sl]),
                "wv": img_w(Wv[:, sl]),
                "wo": wo_img,
                "bqk": bqk_img,
                "bv": np.ascontiguousarray(bv[sl].reshape(1, DHC)),
                "bo": np.ascontiguousarray(
                    (bo if g == 0 else np.zeros_like(bo)).reshape(1, D)
                ),
            }
        )
    return in_maps


def kernel(x, Wq, bq, Wk, bk, Wv, bv, Wo, bo, _trace=False):
    from concourse.bass_utils import run_bass_kernel_spmd

    args = [np.asarray(a, dtype=np.float32) for a in (x, Wq, bq, Wk, bk, Wv, bv, Wo, bo)]
    if "nc" not in _cache:
        _cache["nc"] = _build()
    nc = _cache["nc"]

    in_maps = _pack_inputs(*args)
    res = run_bass_kernel_spmd(nc, in_maps, list(range(NCORE)), trace=_trace)
    _cache["last_result"] = res

    out = np.zeros((B, S, D), dtype=np.float32)
    for i in range(NCORE):
        out[i // TP] += res.results[i]["y"]
    return out
